# revision 1
# baseline (speedup 1.0000x reference)
"""Bass/Trainium2 kernel for nn_BiDirectionalCrossAttentionLayer.

Sharding: 8 cores = batch(4) x head-group(2). Each core computes, for its
batch b and its 4 heads, the full 4-stream cross-attention + the 256 output
rows (t = hg*256 .. hg*256+255) of every stream. The reference's
"transpose(1,2) ... transpose/reshape" scramble maps output row t to
(head t//64, head-dim t%64) over all sequence positions, so a head-split of
attention is exactly an output-row split of everything after it.

All matmuls in bf16 (fp32 accumulate); residuals/LN in fp32.
"""

import os
import numpy as np
import ml_dtypes

import concourse.bacc as bacc
import concourse.bass as bass
import concourse.tile as tile
from concourse import mybir
from concourse.bass_utils import run_bass_kernel_spmd
from concourse.masks import make_identity

BF16 = ml_dtypes.bfloat16
F32 = np.float32

NS, B, S, E, H, HD = 4, 4, 512, 512, 8, 64
SCALE = HD ** -0.5
LN_EPS = 1e-5
P = 128
HG = 2            # head groups == cores per batch
HPC = H // HG // 2  # head-pairs per core = 2
HC = H // HG      # heads per core = 4
TG = S // HG      # output rows per core per stream = 256
TS = TG // P      # row tiles per core = 2
ET = E // P       # embedding tiles = 4
KT = S // P       # key/seq tiles = 4
FT = 4 * E // P   # ffn hidden tiles = 16
N_CORES = B * HG

AF = mybir.ActivationFunctionType
ALU = mybir.AluOpType
AX = mybir.AxisListType
DT_BF = mybir.dt.bfloat16
DT_F32 = mybir.dt.float32


def _build_program(reps=1, phases="all"):
    nc = bacc.Bacc("TRN2", target_bir_lowering=False, debug=False)

    def din(name, shape, dt=DT_BF):
        return nc.dram_tensor(name, list(shape), dt, kind="ExternalInput").ap()

    xT_d = din("xT", (NS, P, ET, S))            # xT[n,p,et,s] = x[n,b,s,et*128+p]
    x32_d = din("x32", (NS, P, TS, E), DT_F32)  # x rows t-slice
    wq_d = din("wq", (NS, P, ET, HC * HD))      # Wq[n, e, hg*256 + c]
    wk_d = din("wk", (NS, P, ET, HC * HD))
    wv_d = din("wv", (NS, P, ET, HC * HD))
    wo_d = din("wo", (NS, P, ET, E))            # Wo[n]/NS, rows e
    w1_d = din("w1", (NS, P, ET, 4 * E))
    w2_d = din("w2", (NS, P, FT, E))
    cmat_d = din("cmat", (P, NS * NS), DT_F32)  # SCALE*inter broadcast on p
    g1_d = din("g1", (NS, E), DT_F32)
    b1_d = din("b1", (NS, E), DT_F32)
    g2_d = din("g2", (NS, E), DT_F32)
    b2_d = din("b2", (NS, E), DT_F32)
    bf1_d = din("bf1", (NS, 4 * E))             # bf1 row (K=1 matmul operand)
    bf2_d = din("bf2", (NS, E), DT_F32)
    out_d = nc.dram_tensor("out", [NS, P, TS, E], DT_F32, kind="ExternalOutput").ap()

    with tile.TileContext(nc) as tc:
        with tc.tile_pool(name="const", bufs=1) as const:
            ident = const.tile([P, P], DT_BF)
            make_identity(nc, ident[:])
            identf = const.tile([P, P], DT_F32)
            make_identity(nc, identf[:])
            cmat_sb = const.tile([P, NS * NS], DT_F32)
            nc.sync.dma_start(cmat_sb[:], cmat_d[:])
            eps_sb = const.tile([P, 1], DT_F32)
            nc.gpsimd.memset(eps_sb[:], LN_EPS)

            # long-lived activations
            r1 = const.tile([P, NS, TS, E], DT_F32)
            r1T = const.tile([P, NS, ET, TG], DT_BF)

          # replicated body via HW loop (reps>1 only for slope timing)
          # fmt: off
            import contextlib
            _loop = tc.For_i(0, reps, 1) if reps > 1 else contextlib.nullcontext()
            with _loop:
              # FFN W1 weight pool first: its DMAs have no deps, so they
              # prefetch during the (otherwise DMA-idle) attention phase.
              f_w1 = tc.alloc_tile_pool(name="f_w1", bufs=3)
              scopeB = tc.alloc_tile_pool(name="scopeB", bufs=1)
              x32 = scopeB.tile([P, NS, TS, E], DT_F32)
              att = scopeB.tile([P, NS, KT, HC * HD], DT_BF)  # att_std accum
              nc.gpsimd.memset(att[:], 0.0)
              for n in range(NS):
                  nc.sync.dma_start(x32[:, n], x32_d[n])

              c_w = tc.alloc_tile_pool(name="c_w", bufs=1)
              wos = c_w.tile([P, NS, ET, E], DT_BF)
              g1b = c_w.tile([P, NS, E], DT_F32)
              b1b = c_w.tile([P, NS, E], DT_F32)
              for n in range(NS):
                  nc.sync.dma_start(wos[:, n], wo_d[n])
                  nc.sync.dma_start(g1b[:, n], g1_d[n].partition_broadcast(P))
                  nc.sync.dma_start(b1b[:, n], b1_d[n].partition_broadcast(P))
              scopeA = tc.alloc_tile_pool(name="scopeA", bufs=1)
              qT = scopeA.tile([P, NS, HPC, S], DT_BF)   # [d-pair rows, n, hp, q]
              kT = scopeA.tile([P, NS, HPC, S], DT_BF)
              vex = scopeA.tile([P, NS, KT, HC, HD + 1], DT_BF)
              nc.gpsimd.memset(vex[:, :, :, :, HD:HD + 1], 1.0)

              # ---------------- Phase 1: QKV projections ----------------
              with tc.tile_pool(name="p1w", bufs=1) as p1w, \
                   tc.tile_pool(name="p1ps", bufs=2, space="PSUM") as p1ps:
                  xTs = p1w.tile([P, NS, ET, S], DT_BF)
                  wqs = p1w.tile([P, NS, ET, HC * HD], DT_BF)
                  wks = p1w.tile([P, NS, ET, HC * HD], DT_BF)
                  wvs = p1w.tile([P, NS, ET, HC * HD], DT_BF)
                  for n in range(NS):
                      nc.sync.dma_start(xTs[:, n], xT_d[n])
                      nc.sync.dma_start(wqs[:, n], wq_d[n])
                      nc.sync.dma_start(wks[:, n], wk_d[n])
                      nc.sync.dma_start(wvs[:, n], wv_d[n])

                  for n in range(NS):
                      for hp in range(HPC):
                          ps_q = p1ps.tile([P, S], DT_F32, tag="psq")
                          ps_k = p1ps.tile([P, S], DT_F32, tag="psk")
                          for et in range(ET):
                              nc.tensor.matmul(
                                  ps_q[:], wqs[:, n, et, hp * P:(hp + 1) * P],
                                  xTs[:, n, et], start=(et == 0), stop=(et == ET - 1))
                          nc.vector.tensor_copy(qT[:, n, hp], ps_q[:])
                          for et in range(ET):
                              nc.tensor.matmul(
                                  ps_k[:], wks[:, n, et, hp * P:(hp + 1) * P],
                                  xTs[:, n, et], start=(et == 0), stop=(et == ET - 1))
                          nc.vector.tensor_copy(kT[:, n, hp], ps_k[:])
                      for kt in range(KT):
                          ps_v = p1ps.tile([P, HC * HD], DT_F32, tag="psv")
                          for et in range(ET):
                              nc.tensor.matmul(
                                  ps_v[:], xTs[:, n, et, kt * P:(kt + 1) * P],
                                  wvs[:, n, et], start=(et == 0), stop=(et == ET - 1))
                          nc.vector.tensor_copy(
                              vex[:, n, kt, :, 0:HD],
                              ps_v[:].rearrange("p (h d) -> p h d", d=HD))

              # FFN W1 weight pool created early: its DMAs have no deps, so
              # they prefetch during the (DMA-idle) attention phase.
              # ---------------- Phase 2: cross-stream attention ----------------
              # Heads of a pair live on disjoint PE row strips (partitions
              # 0-63 / 64-127): issuing their score matmuls back-to-back lets
              # the PE run them concurrently (implicit tile_position row
              # packing from the operands' base partitions).
              with tc.tile_pool(name="a_sps", bufs=1, space="PSUM") as a_sps, \
                   tc.tile_pool(name="a_ups", bufs=2, space="PSUM") as a_ups, \
                   tc.tile_pool(name="a_tps", bufs=2, space="PSUM") as a_tps, \
                   tc.tile_pool(name="a_sb", bufs=4) as a_sb, \
                   tc.tile_pool(name="a_sm", bufs=16) as a_sm:
                  def wo_ln1(i, c_ps, c_tp, c_sb, c_sm):
                      # Wo proj + residual + LayerNorm1 + r1 transpose for
                      # stream i. invstd = exp(-0.5*ln(var/E + eps)) keeps ACT
                      # on the ln/exp table set shared with attention's exps.
                      for ts in range(TS):
                          wo_ps = c_ps.tile([P, E], DT_F32, tag="wops")
                          for qt in range(KT):
                              nc.tensor.matmul(
                                  wo_ps[:], att[:, i, qt, ts * P:(ts + 1) * P],
                                  wos[:, i, qt], start=(qt == 0),
                                  stop=(qt == KT - 1))
                          y1 = c_sb.tile([P, E], DT_F32, tag="y1")
                          nc.vector.tensor_add(y1[:], wo_ps[:], x32[:, i, ts])
                          nm = c_sm.tile([P, 1], DT_F32, tag="nm")
                          nc.vector.reduce_sum(nm[:], y1[:], axis=AX.X)
                          nc.vector.tensor_scalar_mul(nm[:], nm[:], -1.0 / E)
                          xc = c_sb.tile([P, E], DT_F32, tag="xc")
                          nc.vector.tensor_scalar_add(xc[:], y1[:], nm[:])
                          var = c_sm.tile([P, 1], DT_F32, tag="var")
                          sq = c_sb.tile([P, E], DT_F32, tag="sq")
                          nc.vector.scalar_tensor_tensor(
                              out=sq[:], in0=xc[:], scalar=1.0, in1=xc[:],
                              op0=ALU.mult, op1=ALU.mult, accum_out=var[:])
                          inv = c_sm.tile([P, 1], DT_F32, tag="inv")
                          nc.scalar.activation(inv[:], var[:], AF.Ln,
                                               bias=eps_sb[:], scale=1.0 / E)
                          nc.scalar.activation(inv[:], inv[:], AF.Exp,
                                               scale=-0.5)
                          nc.vector.scalar_tensor_tensor(
                              out=r1[:, i, ts], in0=xc[:], scalar=inv[:],
                              in1=g1b[:, i], op0=ALU.mult, op1=ALU.mult)
                          nc.vector.tensor_add(r1[:, i, ts], r1[:, i, ts],
                                               b1b[:, i])
                          for et in range(ET):
                              rt_ps = c_tp.tile([P, P], DT_F32, tag="rt")
                              nc.tensor.transpose(
                                  rt_ps[:], r1[:, i, ts, et * P:(et + 1) * P],
                                  identf[:])
                              nc.vector.tensor_copy(
                                  r1T[:, i, et, ts * P:(ts + 1) * P], rt_ps[:])

                  for i in range(NS if phases in ("all", "attn") else 0):
                      for j in range(NS):
                          c_ap = cmat_sb[:, (i * NS + j):(i * NS + j + 1)]
                          for hp in range(HPC):
                              exs = []
                              for half in range(2):
                                  s1 = a_sps.tile([P, 2, S], DT_F32, tag="s1")
                                  s0 = a_sps.tile([P, 2, S], DT_F32, tag="s0")
                                  for k2 in range(2):
                                      kt = half * 2 + k2
                                      nc.tensor.matmul(
                                          s1[:, k2],
                                          kT[HD:P, j, hp, kt * P:(kt + 1) * P],
                                          qT[HD:P, i, hp],
                                          start=True, stop=True)
                                      nc.tensor.matmul(
                                          s0[:, k2],
                                          kT[0:HD, j, hp, kt * P:(kt + 1) * P],
                                          qT[0:HD, i, hp],
                                          start=True, stop=True)
                                  ex0 = a_sb.tile([P, 2, S], DT_BF, tag="ex0")
                                  ex1 = a_sb.tile([P, 2, S], DT_BF, tag="ex1")
                                  nc.scalar.activation(ex1[:], s1[:], AF.Exp,
                                                       scale=c_ap)
                                  nc.scalar.activation(ex0[:], s0[:], AF.Exp,
                                                       scale=c_ap)
                                  exs.append((ex0, ex1))
                              for sub in range(2):
                                  hl = hp * 2 + sub
                                  ua_ps = a_ups.tile([HD + 1, S], DT_F32,
                                                     tag="ua")
                                  for kt in range(KT):
                                      nc.tensor.matmul(
                                          ua_ps[:], vex[:, j, kt, hl],
                                          exs[kt // 2][sub][:, kt % 2],
                                          start=(kt == 0), stop=(kt == KT - 1))
                                  ua_sb = a_sb.tile([HD + 1, S], DT_BF,
                                                    tag="uasb")
                                  nc.vector.tensor_copy(ua_sb[:], ua_ps[:])
                                  for qt in range(KT):
                                      tr_ps = a_tps.tile([P, HD + 1], DT_BF,
                                                         tag="tr")
                                      nc.tensor.transpose(
                                          tr_ps[:], ua_sb[:, qt * P:(qt + 1) * P],
                                          ident[0:HD + 1, 0:HD + 1])
                                      r_sb = a_sm.tile([P, 1], DT_F32, tag="rr")
                                      nc.vector.reciprocal(r_sb[:],
                                                           tr_ps[:, HD:HD + 1])
                                      nc.vector.scalar_tensor_tensor(
                                          out=att[:, i, qt,
                                                  hl * HD:(hl + 1) * HD],
                                          in0=tr_ps[:, 0:HD], scalar=r_sb[:],
                                          in1=att[:, i, qt,
                                                  hl * HD:(hl + 1) * HD],
                                          op0=ALU.mult, op1=ALU.add)

              scopeA.release()

              # -------- Phase 3: Wo proj + residual + LN1 -------------------
              if phases != "attn":
                  with tc.tile_pool(name="c_ps", bufs=2, space="PSUM") as c_ps, \
                       tc.tile_pool(name="c_tp", bufs=2, space="PSUM") as c_tp, \
                       tc.tile_pool(name="c_sb", bufs=3) as c_sb, \
                       tc.tile_pool(name="c_sm", bufs=6) as c_sm:
                      for i in range(NS):
                          wo_ln1(i, c_ps, c_tp, c_sb, c_sm)

              if phases == "attn":
                  # copy att into the output so DCE cannot drop the attention
                  with tc.tile_pool(name="ao", bufs=2) as ao:
                      for n in range(NS):
                          aout = ao.tile([P, TS, E], DT_F32, tag="aout")
                          nc.vector.tensor_copy(
                              aout[:].rearrange("p a b -> p (a b)"),
                              att[:, n].rearrange("p a b -> p (a b)"))
                          nc.sync.dma_start(out_d[n], aout[:])
              c_w.release()
              scopeB.release()

              # ---------------- Phase 4: FFN W1 + gelu (all streams) ------
              # All gelu before any LN2 sqrt: one ACT table load per set.
              hT_pool = tc.alloc_tile_pool(name="hT_pool", bufs=1)
              hTall = hT_pool.tile([P, NS, FT, TG], DT_BF)
              f_c = tc.alloc_tile_pool(name="f_c", bufs=1)
              bf1r = f_c.tile([1, NS, 4 * E], DT_BF)
              ones_row = f_c.tile([1, TG], DT_BF)
              nc.gpsimd.memset(ones_row[:], 1.0)
              g2b = f_c.tile([P, NS, E], DT_F32)
              b2b = f_c.tile([P, NS, E], DT_F32)
              bf2b = f_c.tile([P, NS, E], DT_F32)
              nc.sync.dma_start(bf1r[:], bf1_d[None, :, :])
              for n in range(NS):
                  nc.sync.dma_start(g2b[:, n], g2_d[n].partition_broadcast(P))
                  nc.sync.dma_start(b2b[:, n], b2_d[n].partition_broadcast(P))
                  nc.sync.dma_start(bf2b[:, n], bf2_d[n].partition_broadcast(P))

              # bf1 enters via a K=1 ones-row matmul so gelu can batch
              # 4 hidden slices per ACTIVATE with no per-slice bias.
              f_w2 = tc.alloc_tile_pool(name="f_w2", bufs=2)
              with tc.tile_pool(name="f_ps", bufs=3, space="PSUM") as f_ps:
                  for n in range(NS if phases != "attn" else 0):
                      w1s = f_w1.tile([P, ET, 4 * E], DT_BF, tag="w1s")
                      nc.sync.dma_start(w1s[:], w1_d[n])
                      for f4 in range(FT // 4):
                          h_ps = f_ps.tile([P, 4, TG], DT_F32, tag="hps")
                          for s4 in range(4):
                              fs = f4 * 4 + s4
                              for et in range(ET):
                                  nc.tensor.matmul(
                                      h_ps[:, s4], w1s[:, et, fs * P:(fs + 1) * P],
                                      r1T[:, n, et], start=(et == 0), stop=False)
                              nc.tensor.matmul(
                                  h_ps[:, s4], bf1r[0:1, n, fs * P:(fs + 1) * P],
                                  ones_row[:], start=False, stop=True)
                          nc.scalar.activation(hTall[:, n, f4 * 4:(f4 + 1) * 4],
                                               h_ps[:], AF.Gelu)

              # ---------------- Phase 5: FFN W2 + residual + LN2 ----------
              with tc.tile_pool(name="f_ps2", bufs=3, space="PSUM") as f_ps2, \
                   tc.tile_pool(name="f_sb", bufs=2) as f_sb, \
                   tc.tile_pool(name="f_sb2", bufs=3) as f_sb2, \
                   tc.tile_pool(name="f_sm", bufs=6) as f_sm:
                  for n in range(NS if phases != "attn" else 0):
                      w2s = f_w2.tile([P, FT, E], DT_BF, tag="w2s")
                      nc.sync.dma_start(w2s[:], w2_d[n])
                      out_sb = f_sb.tile([P, TS, E], DT_F32, tag="outsb")
                      for ts in range(TS):
                          f2_ps = f_ps2.tile([P, E], DT_F32, tag="fps")
                          for ft in range(FT):
                              nc.tensor.matmul(
                                  f2_ps[:], hTall[:, n, ft, ts * P:(ts + 1) * P],
                                  w2s[:, ft], start=(ft == 0), stop=(ft == FT - 1))
                          y2 = f_sb2.tile([P, E], DT_F32, tag="y2")
                          nc.vector.tensor_add(y2[:], f2_ps[:], bf2b[:, n])
                          nc.vector.tensor_add(y2[:], y2[:], r1[:, n, ts])
                          # LayerNorm 2
                          nm = f_sm.tile([P, 1], DT_F32, tag="nm2")
                          nc.vector.reduce_sum(nm[:], y2[:], axis=AX.X)
                          nc.vector.tensor_scalar_mul(nm[:], nm[:], -1.0 / E)
                          xc = f_sb2.tile([P, E], DT_F32, tag="xc2")
                          nc.vector.tensor_scalar_add(xc[:], y2[:], nm[:])
                          var = f_sm.tile([P, 1], DT_F32, tag="var2")
                          sq = f_sb2.tile([P, E], DT_F32, tag="sq2")
                          nc.vector.scalar_tensor_tensor(
                              out=sq[:], in0=xc[:], scalar=1.0, in1=xc[:],
                              op0=ALU.mult, op1=ALU.mult, accum_out=var[:])
                          inv = f_sm.tile([P, 1], DT_F32, tag="inv2")
                          nc.scalar.activation(inv[:], var[:], AF.Sqrt,
                                               bias=eps_sb[:], scale=1.0 / E)
                          nc.vector.reciprocal(inv[:], inv[:])
                          nc.vector.scalar_tensor_tensor(
                              out=out_sb[:, ts], in0=xc[:], scalar=inv[:],
                              in1=g2b[:, n], op0=ALU.mult, op1=ALU.mult)
                          nc.vector.tensor_add(out_sb[:, ts], out_sb[:, ts],
                                               b2b[:, n])
                      nc.sync.dma_start(out_d[n], out_sb[:])
              f_w2.release()
              f_c.release()
              hT_pool.release()
              f_w1.release()

    nc.compile()
    return nc


_NC_CACHE = {}


def _get_nc(reps=1, phases="all"):
    key = f"nc{reps}_{phases}"
    if key not in _NC_CACHE:
        _NC_CACHE[key] = _build_program(reps, phases)
    return _NC_CACHE[key]


def _pack_inputs(x0, x1, x2, x3, Wq, Wk, Wv, Wo, bo, ln1_g, ln1_b, ln2_g, ln2_b,
                 W1, bf1, W2, bf2, inter):
    x = np.stack([np.asarray(x0), np.asarray(x1), np.asarray(x2),
                  np.asarray(x3)]).astype(F32)  # [NS,B,S,E]
    Wq, Wk, Wv, Wo = (np.asarray(a, F32) for a in (Wq, Wk, Wv, Wo))
    inputs_bo = np.asarray(bo, F32)
    W1, W2 = np.asarray(W1, F32), np.asarray(W2, F32)
    inter = np.asarray(inter, F32)

    def tile_rows(a, nt):
        # [NS, R, C] -> [NS, P, nt, C]
        return np.ascontiguousarray(
            a.reshape(NS, nt, P, a.shape[-1]).transpose(0, 2, 1, 3))

    shared = {
        "wo": tile_rows(Wo / NS, ET).astype(BF16),
        "w1": tile_rows(W1, ET).astype(BF16),
        "w2": tile_rows(W2, FT).astype(BF16),
        "cmat": np.ascontiguousarray(
            np.broadcast_to((inter * SCALE).reshape(1, NS * NS), (P, NS * NS))
        ).astype(F32),
        "g1": np.ascontiguousarray(ln1_g, dtype=F32),
        "b1": np.ascontiguousarray(ln1_b, dtype=F32),
        "g2": np.ascontiguousarray(ln2_g, dtype=F32),
        "b2": np.ascontiguousarray(ln2_b, dtype=F32),
        "bf1": np.ascontiguousarray(np.asarray(bf1, F32)).astype(BF16),
        "bf2": np.ascontiguousarray(bf2, dtype=F32),
    }
    per_hg = []
    for hg in range(HG):
        cols = slice(hg * HC * HD, (hg + 1) * HC * HD)
        per_hg.append({
            "wq": tile_rows(Wq[:, :, cols], ET).astype(BF16),
            "wk": tile_rows(Wk[:, :, cols], ET).astype(BF16),
            "wv": tile_rows(Wv[:, :, cols], ET).astype(BF16),
        })
    in_maps = []
    for core in range(N_CORES):
        b, hg = core // HG, core % HG
        xb = x[:, b]  # [NS, S, E]
        xT = np.ascontiguousarray(
            xb.transpose(0, 2, 1).reshape(NS, ET, P, S).transpose(0, 2, 1, 3)
        ).astype(BF16)
        x32 = np.ascontiguousarray(
            (xb[:, hg * TG:(hg + 1) * TG] + np.asarray(
                inputs_bo)[:, None, :]).reshape(NS, TS, P, E)
            .transpose(0, 2, 1, 3).astype(F32))
        m = {"xT": xT, "x32": x32}
        m.update(shared)
        m.update(per_hg[hg])
        in_maps.append(m)
    return in_maps


def _unpack_outputs(results):
    full = np.empty((NS, B, S, E), dtype=F32)
    for core in range(N_CORES):
        b, hg = core // HG, core % HG
        o = results[core]["out"]  # [NS, P, TS, E]
        full[:, b, hg * TG:(hg + 1) * TG] = (
            o.transpose(0, 2, 1, 3).reshape(NS, TG, E))
    return tuple(full[n] for n in range(NS))


def kernel(**inputs):
    nc = _get_nc()
    in_maps = _pack_inputs(**inputs)
    res = run_bass_kernel_spmd(
        nc, in_maps, core_ids=list(range(N_CORES)),
        trace=bool(int(os.environ.get("KERNEL_TRACE", "0"))))
    _NC_CACHE["last_result"] = res
    return _unpack_outputs(res.results)


def bench(inputs, iters=20, reps=1, phases="all"):
    """Time the on-device execution with device-resident inputs.

    Mirrors bass2jax.run_bass_via_pjrt's shard_map(_bass_exec) lowering but
    without output-buffer donation, so the same executable can be re-invoked
    and timed. Returns (min, median) seconds per call. With reps>1 the NEFF
    contains the kernel body replicated; use slopes across reps to cancel
    the fixed axon-RPC dispatch overhead.
    """
    import time
    import jax
    import jax.numpy as jnp
    from jax.sharding import Mesh, PartitionSpec, NamedSharding
    from jax.experimental.shard_map import shard_map
    from concourse import bass2jax
    from concourse import mybir as mb

    nc = _get_nc(reps, phases)
    bass2jax.install_neuronx_cc_hook()
    in_maps = _pack_inputs(**inputs)

    part_name = nc.partition_id_tensor.name if nc.partition_id_tensor else None
    in_names, out_names, out_avals, zero_outs = [], [], [], []
    for alloc in nc.m.functions[0].allocations:
        if not isinstance(alloc, mb.MemoryLocationSet):
            continue
        name = alloc.memorylocations[0].name
        if alloc.kind == "ExternalInput":
            if name != part_name:
                in_names.append(name)
        elif alloc.kind == "ExternalOutput":
            out_names.append(name)
            shape = tuple(alloc.tensor_shape)
            dtype = mb.dt.np(alloc.dtype)
            out_avals.append(jax.core.ShapedArray(shape, dtype))
            zero_outs.append(np.zeros(shape, dtype))
    n_params = len(in_names)
    all_names = in_names + out_names
    if part_name is not None:
        all_names = all_names + [part_name]

    def _body(*args):
        operands = list(args)
        if part_name is not None:
            operands.append(bass2jax.partition_id_tensor())
        outs = bass2jax._bass_exec_p.bind(
            *operands, out_avals=tuple(out_avals), in_names=tuple(all_names),
            out_names=tuple(out_names), lowering_input_output_aliases=(),
            sim_require_finite=True, sim_require_nnan=True, nc=nc)
        return tuple(outs)

    devices = jax.devices()[:N_CORES]
    mesh = Mesh(np.asarray(devices), ("core",))
    spec = PartitionSpec("core")
    fn = jax.jit(shard_map(
        _body, mesh=mesh, in_specs=(spec,) * (n_params + len(out_names)),
        out_specs=(spec,) * len(out_names), check_rep=False))
    sh = NamedSharding(mesh, spec)
    concat = [jax.device_put(
        np.concatenate([in_maps[c][nm] for c in range(N_CORES)], axis=0), sh)
        for nm in in_names]
    concat += [jax.device_put(
        np.zeros((N_CORES * z.shape[0], *z.shape[1:]), z.dtype), sh)
        for z in zero_outs]

    out = fn(*concat)  # compile
    jax.block_until_ready(out)
    times = []
    for _ in range(iters):
        t0 = time.perf_counter()
        out = fn(*concat)
        jax.block_until_ready(out)
        times.append(time.perf_counter() - t0)
    times.sort()
    return times[0], times[len(times) // 2]


if __name__ == "__main__":
    import sys
    mode = sys.argv[1] if len(sys.argv) > 1 else "sim"
    sys.path.insert(0, os.path.dirname(os.path.abspath(__file__)))
    import reference

    inputs = {k: np.asarray(v) for k, v in reference.setup_inputs().items()}
    if mode == "sim":
        # Simulate core 0 (b=0, hg=0) with CoreSim and compare to reference.
        # CoreSim has no Gelu; patch exact erf-gelu into its activation visitor.
        import concourse.bass_interp as bass_interp
        from scipy.special import erf as _erf
        _orig_visit = bass_interp.InstructionExecutor.visit_InstActivation

        def _patched(self, instruction, reg_snapshot=None):
            if instruction.func == mybir.ActivationFunctionType.Gelu:
                instruction.func = mybir.ActivationFunctionType.Identity
                try:
                    import concourse.mybir as mb
                    from concourse.bass_interp import Direction
                    out_ap = instruction.outs[0]
                    res = _orig_visit(self, instruction, reg_snapshot=reg_snapshot)
                    v = self.view_ap(out_ap, Direction.WRITE, instruction,
                                     reg_snapshot=reg_snapshot)
                    x = v[:].astype(np.float32)
                    v[:] = (x * 0.5 * (1.0 + _erf(x / np.sqrt(2.0)))).astype(v.dtype)
                    return res
                finally:
                    instruction.func = mybir.ActivationFunctionType.Gelu
            return _orig_visit(self, instruction, reg_snapshot=reg_snapshot)

        bass_interp.InstructionExecutor.visit_InstActivation = _patched
        from concourse.bass_interp import CoreSim
        nc = _get_nc()
        in_maps = _pack_inputs(**inputs)
        sim = CoreSim(nc, trace=False)
        for name, arr in in_maps[0].items():
            sim.tensor(name)[:] = arr
        sim.simulate(check_with_hw=False)
        out = sim.tensor("out").copy()
        got = out.transpose(0, 2, 1, 3).reshape(NS, TG, E)
        exp = np.stack([np.asarray(o) for o in reference.reference(**inputs)])
        exp_slice = exp[:, 0, 0:TG]  # b=0, rows 0:256
        err = np.abs(got - exp_slice)
        rel = np.linalg.norm(got - exp_slice) / np.linalg.norm(exp_slice)
        print(f"max abs err: {err.max():.3e}  rel fro err: {rel:.3e}")
    else:
        got = kernel(**inputs)
        exp = reference.reference(**inputs)
        for n in range(NS):
            g, e = np.asarray(got[n]), np.asarray(exp[n])
            rel = np.linalg.norm(g - e) / np.linalg.norm(e)
            print(f"out{n}: rel fro err {rel:.3e} max abs {np.abs(g - e).max():.3e}")

